# revision 20
# baseline (speedup 1.0000x reference)
"""AWQ W4A16 Linear (out = x @ dequant(qweight) + bias) on 8 TRN2 NeuronCores.

Tensor-parallel over out_features: each core owns a contiguous slice of
N = 12288 (1536 columns), dequantizes its int4 weight shard on-chip into a
SBUF-resident bf16 [K, N_local] matrix, and runs a PE-bound matmul over the
replicated activations.

Per-core pipeline:
  phase 1 (dequant):  qweight int32 [384, 2048] -> (shift,mask) nibble planes
                      -> (q - z) * s in bf16 -> W_pre [n_local, K] staged to a
                      DRAM scratch (rows written n-natural via strided DMA)
  phase 2 (matmul):   W_pre --dma_start_transpose--> W' tiles [128 k, 1536 n]
                      resident in SBUF; x panels transposed on load
                      ([512 m, 128 k] -> [128 k, 512 m]); psum [128 m, 512 n]
                      accumulated over 32 k-tiles; evict = cast bf16 + bias.
"""

import numpy as np
import ml_dtypes
from contextlib import ExitStack

import concourse.bass as bass
import concourse.bacc as bacc
import concourse.mybir as mybir
import concourse.tile as tile
from concourse.bass_utils import run_bass_kernel_spmd

BF16 = mybir.dt.bfloat16
I32 = mybir.dt.int32
F32 = mybir.dt.float32

M_FULL = 4096
K = 4096
N_FULL = 12288
N_CORES = 8
NL = N_FULL // N_CORES          # 1536 out features per core
GS = 64                         # quant group size
NG = K // GS                    # 64 groups
NKT = K // 128                  # 32 k-tiles
PANEL = 512                     # m-panel size
NB = NL // 512                  # 3 psum n-blocks per core


def build_nc(m: int = M_FULL, debug_taps: bool = False, phases: str = "all") -> bass.Bass:
    nc = bacc.Bacc(None)
    x = nc.dram_tensor("x", [m, K], BF16, kind="ExternalInput")
    qw = nc.dram_tensor("qw", [NL // 4, K // 2], I32, kind="ExternalInput")
    # st/zt[t, r, g] = wscales/wzeros[g, n0 + 4*t + r]  (host pre-arranged)
    st = nc.dram_tensor("st", [NL // 4, 4, NG], BF16, kind="ExternalInput")
    zt = nc.dram_tensor("zt", [NL // 4, 4, NG], I32, kind="ExternalInput")
    bias = nc.dram_tensor("bias", [NL], BF16, kind="ExternalInput")
    out = nc.dram_tensor("out", [m, NL], BF16, kind="ExternalOutput")

    n_mp = m // PANEL
    n_ms = PANEL // 128

    with tile.TileContext(nc) as tc, ExitStack() as ctx:
        dram = ctx.enter_context(tc.tile_pool(name="dram", bufs=1, space="DRAM"))
        wpre = dram.tile([NL, K], BF16, name="wpre", uniquify=False)
        # wpre viewed so row n = 4*t + r is addressed as [r, t]
        wpre_rt = wpre[:].rearrange("(t four) k -> four t k", four=4)

        const = ctx.enter_context(tc.tile_pool(name="const", bufs=1))
        bias_sb = const.tile([128, NL], BF16)
        bias_bc = bass.AP(
            tensor=bias[:].tensor, offset=bias[:].offset, ap=[[0, 128], [1, NL]]
        )
        nc.gpsimd.dma_start(out=bias_sb[:], in_=bias_bc)

        st_sb = []
        zt_sb = []
        for t3 in range(3):
            s_t = const.tile([128, 4, NG], BF16, name=f"st{t3}")
            z_t = const.tile([128, 4, NG], I32, name=f"zt{t3}")
            nc.sync.dma_start(out=s_t[:], in_=st[t3 * 128:(t3 + 1) * 128])
            nc.sync.dma_start(out=z_t[:], in_=zt[t3 * 128:(t3 + 1) * 128])
            st_sb.append(s_t)
            zt_sb.append(z_t)

        # ---- phase 1: dequant into wpre (DRAM scratch) ----
        if phases in ("all", "deq"):
            _build_phase1(nc, tc, qw, st, zt, wpre_rt, st_sb, zt_sb)
        if phases in ("all", "mm"):
            _build_phase2(nc, tc, ctx, x, out, wpre, bias_sb, m, n_mp, n_ms,
                          debug_taps)
    nc.compile()
    return nc


def _build_phase1(nc, tc, qw, st, zt, wpre_rt, st_sb, zt_sb):
        with tc.tile_pool(name="deq", bufs=2) as deq, \
             tc.tile_pool(name="qwp", bufs=3) as qwp, \
             tc.tile_pool(name="wprep", bufs=2) as wprep:
            qw_sb = []
            for t3 in range(3):
                q_t = qwp.tile([128, K // 2], I32, tag="qw")
                nc.sync.dma_start(out=q_t[:], in_=qw[t3 * 128:(t3 + 1) * 128])
                qw_sb.append(q_t)

            for r in range(4):
                for t3 in range(3):
                    wp_t = wprep.tile([128, K], BF16, tag="wp")
                    for c in range(2):
                        j = 2 * r + c
                        nib = deq.tile([128, K // 2], I32, tag="nib")
                        nc.vector.tensor_scalar(
                            nib[:], qw_sb[t3][:], 4 * j, 0xF,
                            mybir.AluOpType.logical_shift_right,
                            mybir.AluOpType.bitwise_and,
                        )
                        diff = deq.tile([128, K // 2], BF16, tag="diff")
                        nib_g = nib[:].rearrange("p (g q) -> p g q", q=GS // 2)
                        diff_g = diff[:].rearrange("p (g q) -> p g q", q=GS // 2)
                        z_bc = zt_sb[t3][:, r, :, None].broadcast_to(
                            [128, NG, GS // 2]
                        )
                        s_bc = st_sb[t3][:, r, :, None].broadcast_to(
                            [128, NG, GS // 2]
                        )
                        nc.vector.tensor_tensor(
                            diff_g, nib_g, z_bc, mybir.AluOpType.subtract
                        )
                        # k = GS*g + 2*u + c
                        wp_view = wp_t[:].rearrange(
                            "p (g u two) -> p two g u", two=2, u=GS // 2
                        )[:, c]
                        nc.vector.tensor_tensor(
                            wp_view, diff_g, s_bc, mybir.AluOpType.mult
                        )
                    nc.sync.dma_start(
                        out=wpre_rt[r, t3 * 128:(t3 + 1) * 128], in_=wp_t[:]
                    )


def _build_phase2(nc, tc, ctx, x, out, wpre, bias_sb, m, n_mp, n_ms, debug_taps):
        # ---- phase 2: transpose-load W', stream x, matmul ----
        wqp = ctx.enter_context(tc.tile_pool(name="wqp", bufs=NKT))
        wq = []
        for kt in range(NKT):
            w_t = wqp.tile([128, NL], BF16, tag="wq")
            nc.sync.dma_start(
                out=w_t[:], in_=wpre[:, kt * 128:(kt + 1) * 128], transpose=True
            )
            wq.append(w_t)

        xpp = ctx.enter_context(tc.tile_pool(name="xpp", bufs=2))
        psp = ctx.enter_context(tc.tile_pool(name="psp", bufs=6, space="PSUM"))
        outp = ctx.enter_context(tc.tile_pool(name="outp", bufs=3))

        if debug_taps:
            dbg_wq = nc.dram_tensor("dbg_wq", [128, NL], BF16, kind="ExternalOutput")
            nc.sync.dma_start(out=dbg_wq[:], in_=wq[0][:])
            dbg_xp = nc.dram_tensor("dbg_xp", [128, PANEL], BF16, kind="ExternalOutput")

        for mp in range(n_mp):
            xp_t = xpp.tile([128, NKT, PANEL], BF16, tag="xp")
            for kt in range(NKT):
                nc.sync.dma_start(
                    out=xp_t[:, kt, :],
                    in_=x[mp * PANEL:(mp + 1) * PANEL, kt * 128:(kt + 1) * 128],
                    transpose=True,
                )
            if debug_taps and mp == 0:
                nc.sync.dma_start(out=dbg_xp[:], in_=xp_t[:, 0, :])
            for ms in range(n_ms):
                out_t = outp.tile([128, NL], BF16, tag="out")
                pss = [psp.tile([128, 512], F32, tag="ps", name=f"ps{nb}")
                       for nb in range(NB)]
                # kt outer / nb inner: 3 consecutive matmuls share the same
                # stationary lhsT (the PE skips redundant weight reloads)
                for kt in range(NKT):
                    for nb in range(NB):
                        nc.tensor.matmul(
                            pss[nb][:],
                            lhsT=xp_t[:, kt, ms * 128:(ms + 1) * 128],
                            rhs=wq[kt][:, nb * 512:(nb + 1) * 512],
                            start=(kt == 0),
                            stop=(kt == NKT - 1),
                        )
                for nb in range(NB):
                    o_slice = out_t[:, nb * 512:(nb + 1) * 512]
                    nc.vector.tensor_copy(o_slice, pss[nb][:])
                    nc.vector.tensor_tensor(
                        o_slice, o_slice,
                        bias_sb[:, nb * 512:(nb + 1) * 512],
                        mybir.AluOpType.add,
                    )
                nc.sync.dma_start(
                    out=out[mp * PANEL + ms * 128:mp * PANEL + (ms + 1) * 128],
                    in_=out_t[:],
                )


def shard_inputs(x, qweight, wscales, wzeros, bias):
    """Split the full problem into per-core input maps."""
    in_maps = []
    x = np.ascontiguousarray(x)
    for i in range(N_CORES):
        n0 = i * NL
        qw_s = np.ascontiguousarray(qweight[n0 // 4:(n0 + NL) // 4])
        s_s = np.ascontiguousarray(wscales[:, n0:n0 + NL].T).reshape(NL // 4, 4, NG)
        z_s = np.ascontiguousarray(
            wzeros[:, n0:n0 + NL].T.astype(np.int32)).reshape(NL // 4, 4, NG)
        b_s = np.ascontiguousarray(bias[n0:n0 + NL])
        in_maps.append({"x": x, "qw": qw_s, "st": s_s, "zt": z_s, "bias": b_s})
    return in_maps


_CACHED_NC = None


def kernel(x, qweight, wscales, wzeros, bias):
    global _CACHED_NC
    x = np.asarray(x, dtype=ml_dtypes.bfloat16)
    qweight = np.asarray(qweight, dtype=np.int32)
    wscales = np.asarray(wscales, dtype=ml_dtypes.bfloat16)
    wzeros = np.asarray(wzeros, dtype=ml_dtypes.bfloat16)
    bias = np.asarray(bias, dtype=ml_dtypes.bfloat16)

    if _CACHED_NC is None:
        _CACHED_NC = build_nc(M_FULL)
    nc = _CACHED_NC
    in_maps = shard_inputs(x, qweight, wscales, wzeros, bias)
    res = run_bass_kernel_spmd(nc, in_maps, list(range(N_CORES)))
    outs = [res.results[i]["out"] for i in range(N_CORES)]
    return np.concatenate(outs, axis=1)


# revision 64
# speedup vs baseline: 4.2850x; 4.2850x over previous
"""AWQ W4A16 Linear (out = x @ dequant(qweight) + bias) on 8 TRN2 NeuronCores.

Tensor-parallel over out_features: each core owns a contiguous slice of
N = 12288 (1536 columns), dequantizes its int4 weight shard on-chip into a
SBUF-resident bf16 [K, N_local] matrix, and runs a PE-bound matmul over the
replicated activations. No collectives; the host concatenates the 8 column
slices.

Per-core pipeline (single Bass program, graduated k-chunks CH_KT so the PE
can start while dequant is still running):
  dequant (DVE):   qweight int32 [384, i-chunk] -> fused (shift & 0xF)
                   tensor_scalar -> (q - z) exact in int -> * scale (bf16,
                   parity-strided write) -> W_pre rows staged to per-chunk
                   DRAM scratch (row n = 4t + r written n-naturally)
  W' load (SP):    one dma_start_transpose per chunk:
                   [1536 n, k-chunk] -> [128 k, kt, 1536 n], SBUF-resident
  x stream (ACT):  one xbar-transpose DMA per 512-row panel:
                   [512 m, 4096 k] -> [128 k, 32 kt, 512 m], prefetch depth 1
  matmul (PE):     psum [128 m, 512 n] over 32 k-tiles; kt-outer/nb-inner so
                   consecutive matmuls share the stationary lhsT; 8 psum banks
  evict:           ACT copy psum->bf16 (rounds once, matching the reference),
                   DVE adds bias, contiguous store per m-block.
"""

import numpy as np
import ml_dtypes
from contextlib import ExitStack

import concourse.bass as bass
import concourse.bacc as bacc
import concourse.mybir as mybir
import concourse.tile as tile
from concourse.bass_utils import run_bass_kernel_spmd

BF16 = mybir.dt.bfloat16
I32 = mybir.dt.int32
F32 = mybir.dt.float32

M_FULL = 4096
K = 4096
N_FULL = 12288
N_CORES = 8
NL = N_FULL // N_CORES          # 1536 out features per core
GS = 64                         # quant group size
NG = K // GS                    # 64 groups
NKT = K // 128                  # 32 k-tiles
PANEL = 512                     # m-panel size
NB = NL // 512                  # 3 psum n-blocks per core
# dequant k-chunks (pipelines W' availability); graduated so the first
# W' tiles reach the PE quickly: sizes are in k-tiles (128 k each)
CH_KT = [2, 2, 4, 8, 8, 8]
NCH = len(CH_KT)
CH_KT0 = [sum(CH_KT[:i]) for i in range(NCH)]   # start k-tile per chunk


def build_nc(m: int = M_FULL, phases: str = "all",
             repeat: int = 1, debug_taps: bool = False) -> bass.Bass:
    nc = bacc.Bacc(None)
    x = nc.dram_tensor("x", [m, K], BF16, kind="ExternalInput")
    qw = nc.dram_tensor("qw", [NL // 4, K // 2], I32, kind="ExternalInput")
    # st/zt[t, r, g] = wscales/wzeros[g, n0 + 4*t + r]  (host pre-arranged)
    st = nc.dram_tensor("st", [NL // 4, 4, NG], BF16, kind="ExternalInput")
    zt = nc.dram_tensor("zt", [NL // 4, 4, NG], I32, kind="ExternalInput")
    bias = nc.dram_tensor("bias", [NL], BF16, kind="ExternalInput")
    out = nc.dram_tensor("out", [m, NL], BF16, kind="ExternalOutput")

    n_mp = m // PANEL
    n_ms = PANEL // 128

    with tile.TileContext(nc) as tc, ExitStack() as ctx:
        dram = ctx.enter_context(tc.tile_pool(name="dram", bufs=1, space="DRAM"))
        # one DRAM staging tile per k-chunk so Tile's per-tile dependency
        # tracking lets W' k-tiles of finished chunks load early
        wpre_ch = []
        for ch in range(NCH):
            w_c = dram.tile([NL, CH_KT[ch] * 128], BF16, name=f"wpre{ch}",
                            uniquify=False)
            wpre_ch.append(w_c)

        const = ctx.enter_context(tc.tile_pool(name="const", bufs=1))
        bias_sb = const.tile([128, NL], BF16)
        bias_bc = bass.AP(
            tensor=bias[:].tensor, offset=bias[:].offset, ap=[[0, 128], [1, NL]]
        )
        nc.gpsimd.dma_start(out=bias_sb[:], in_=bias_bc)

        st_sb = []
        zt_sb = []
        for t3 in range(3):
            s_t = const.tile([128, 4, NG], BF16, name=f"st{t3}")
            z_t = const.tile([128, 4, NG], I32, name=f"zt{t3}")
            nc.sync.dma_start(out=s_t[:], in_=st[t3 * 128:(t3 + 1) * 128])
            nc.sync.dma_start(out=z_t[:], in_=zt[t3 * 128:(t3 + 1) * 128])
            st_sb.append(s_t)
            zt_sb.append(z_t)

        # ---- pipeline body ----
        for rep in range(repeat):
            _build_pipeline(nc, tc, qw, x, out, wpre_ch, st_sb, zt_sb, bias_sb,
                            m, n_mp, n_ms, phases)
    nc.compile()
    return nc


def _build_pipeline(nc, tc, qw, x, out, wpre_ch, st_sb, zt_sb, bias_sb,
                    m, n_mp, n_ms, phases):
      with ExitStack() as ctx:
        deq = ctx.enter_context(tc.tile_pool(name="deq", bufs=2))
        qwp = ctx.enter_context(tc.tile_pool(name="qwp", bufs=1))
        wprep = ctx.enter_context(tc.tile_pool(name="wprep", bufs=2))
        wqp = ctx.enter_context(tc.tile_pool(name="wqp", bufs=1))
        xpp = ctx.enter_context(tc.tile_pool(name="xpp", bufs=2))
        psp = ctx.enter_context(tc.tile_pool(name="psp", bufs=8, space="PSUM"))
        outp = ctx.enter_context(tc.tile_pool(name="outp", bufs=2))

        do_deq = phases in ("all", "deq")
        do_mm = phases in ("all", "mm")

        # x panel transpose-loads: issued on the ACT HWDGE ring so they are
        # not stuck behind the dequant staging traffic on the SP ring.
        # Only panel 0 is queued upfront; panel i+1 is queued when panel i's
        # matmuls are emitted, so early x traffic doesn't delay the first
        # W' chunk on the shared DMA engines.
        xp_tiles = []

        def load_panel(mp):
            xp_t = xpp.tile([128, NKT, PANEL], BF16, tag="xp", name=f"xp{mp}")
            # whole panel in one xbar-transpose DMA:
            # [PANEL, K] -> [128, NKT, PANEL] (out[:, e, :] = cols 128e..)
            nc.scalar.dma_start(
                out=xp_t[:],
                in_=x[mp * PANEL:(mp + 1) * PANEL, :],
                transpose=True,
            )
            xp_tiles.append(xp_t)

        if do_mm:
            load_panel(0)

        wq = [None] * NKT
        if do_deq:
            for ch in range(NCH):
                ich = CH_KT[ch] * 64        # packed int32 cols in this chunk
                i0 = CH_KT0[ch] * 64
                gch = CH_KT[ch] * 2         # 64-k groups in this chunk
                g0 = CH_KT0[ch] * 2
                # qweight shard k-chunk in one DMA: [384, ich] -> [128, 3, ich]
                qw_full = qwp.tile([128, 3, max(CH_KT) * 64], I32, tag="qw",
                                   name=f"qwb{ch}")
                qw_big = qw_full[:, :, :ich]
                nc.sync.dma_start(
                    out=qw_big,
                    in_=qw[:, i0:i0 + ich].rearrange("(t3 p) i -> p t3 i", p=128),
                )
                # wpre_ch[ch] viewed so row n = 4*t + r is addressed [r, t]
                w_rt = wpre_ch[ch][:].rearrange("(t four) k -> four t k", four=4)
                for r in range(4):
                    for t3 in range(3):
                        wp_full = wprep.tile([128, max(CH_KT) * 128], BF16,
                                             tag="wp", name=f"wp{ch}_{r}_{t3}")
                        wp_t = wp_full[:, :CH_KT[ch] * 128]
                        for c in range(2):
                            j = 2 * r + c
                            nib_full = deq.tile([128, max(CH_KT) * 64], I32,
                                                tag="nib", name=f"nib{ch}_{j}")
                            nib = nib_full[:, :ich]
                            nc.vector.tensor_scalar(
                                nib,
                                qw_big[:, t3, :],
                                4 * j, 0xF,
                                mybir.AluOpType.logical_shift_right,
                                mybir.AluOpType.bitwise_and,
                            )
                            diff_full = deq.tile([128, max(CH_KT) * 64], BF16,
                                                 tag="diff", name=f"diff{ch}_{j}")
                            diff = diff_full[:, :ich]
                            nib_g = nib.rearrange("p (g q) -> p g q", q=GS // 2)
                            diff_g = diff.rearrange("p (g q) -> p g q", q=GS // 2)
                            z_bc = zt_sb[t3][
                                :, r, g0:g0 + gch, None
                            ].broadcast_to([128, gch, GS // 2])
                            s_bc = st_sb[t3][
                                :, r, g0:g0 + gch, None
                            ].broadcast_to([128, gch, GS // 2])
                            nc.vector.tensor_tensor(
                                diff_g, nib_g, z_bc, mybir.AluOpType.subtract
                            )
                            # k_local = GS*g + 2*u + c
                            wp_view = wp_t.rearrange(
                                "p (g u two) -> p two g u", two=2, u=GS // 2
                            )[:, c]
                            nc.vector.tensor_tensor(
                                wp_view, diff_g, s_bc, mybir.AluOpType.mult
                            )
                        nc.sync.dma_start(
                            out=w_rt[r, t3 * 128:(t3 + 1) * 128], in_=wp_t
                        )
                if do_mm:
                    # all W' k-tiles of this chunk in ONE transpose DMA,
                    # right behind the chunk's stores on the SP ring
                    w_t = wqp.tile([128, CH_KT[ch], NL], BF16, tag=f"wq{ch}",
                                   name=f"wqc{ch}")
                    nc.sync.dma_start(
                        out=w_t[:], in_=wpre_ch[ch][:], transpose=True
                    )
                    for kt in range(CH_KT0[ch], CH_KT0[ch] + CH_KT[ch]):
                        wq[kt] = w_t[:, kt - CH_KT0[ch]]
        elif do_mm:
            for ch in range(NCH):
                w_t = wqp.tile([128, CH_KT[ch], NL], BF16, tag=f"wq{ch}",
                               name=f"wqc{ch}")
                nc.sync.dma_start(
                    out=w_t[:], in_=wpre_ch[ch][:], transpose=True
                )
                for kt in range(CH_KT0[ch], CH_KT0[ch] + CH_KT[ch]):
                    wq[kt] = w_t[:, kt - CH_KT0[ch]]

        if not do_mm:
            return
        for mp in range(n_mp):
            if mp + 1 < n_mp:
                load_panel(mp + 1)
            xp_t = xp_tiles[mp]
            for ms in range(n_ms):
                out_t = outp.tile([128, NL], BF16, tag="out")
                pss = [psp.tile([128, 512], F32, tag="ps", name=f"ps{nb}")
                       for nb in range(NB)]
                # kt outer / nb inner: 3 consecutive matmuls share the same
                # stationary lhsT (the PE skips redundant weight reloads)
                for kt in range(NKT):
                    for nb in range(NB):
                        nc.tensor.matmul(
                            pss[nb][:],
                            lhsT=xp_t[:, kt, ms * 128:(ms + 1) * 128],
                            rhs=wq[kt][:, nb * 512:(nb + 1) * 512],
                            start=(kt == 0),
                            stop=(kt == NKT - 1),
                        )
                for nb in range(NB):
                    o_slice = out_t[:, nb * 512:(nb + 1) * 512]
                    # psum -> sbuf bf16 cast on the (otherwise idle) ACT engine
                    nc.scalar.activation(
                        o_slice, pss[nb][:], mybir.ActivationFunctionType.Copy
                    )
                    nc.vector.tensor_tensor(
                        o_slice, o_slice,
                        bias_sb[:, nb * 512:(nb + 1) * 512],
                        mybir.AluOpType.add,
                    )
                nc.sync.dma_start(
                    out=out[mp * PANEL + ms * 128:mp * PANEL + (ms + 1) * 128],
                    in_=out_t[:],
                )


def shard_inputs(x, qweight, wscales, wzeros, bias):
    """Split the full problem into per-core input maps."""
    in_maps = []
    x = np.ascontiguousarray(x)
    for i in range(N_CORES):
        n0 = i * NL
        qw_s = np.ascontiguousarray(qweight[n0 // 4:(n0 + NL) // 4])
        s_s = np.ascontiguousarray(wscales[:, n0:n0 + NL].T).reshape(NL // 4, 4, NG)
        z_s = np.ascontiguousarray(
            wzeros[:, n0:n0 + NL].T.astype(np.int32)).reshape(NL // 4, 4, NG)
        b_s = np.ascontiguousarray(bias[n0:n0 + NL])
        in_maps.append({"x": x, "qw": qw_s, "st": s_s, "zt": z_s, "bias": b_s})
    return in_maps


_CACHED_NC = None


def kernel(x, qweight, wscales, wzeros, bias):
    global _CACHED_NC
    x = np.asarray(x, dtype=ml_dtypes.bfloat16)
    qweight = np.asarray(qweight, dtype=np.int32)
    wscales = np.asarray(wscales, dtype=ml_dtypes.bfloat16)
    wzeros = np.asarray(wzeros, dtype=ml_dtypes.bfloat16)
    bias = np.asarray(bias, dtype=ml_dtypes.bfloat16)

    if _CACHED_NC is None:
        _CACHED_NC = build_nc(M_FULL)
    nc = _CACHED_NC
    in_maps = shard_inputs(x, qweight, wscales, wzeros, bias)
    res = run_bass_kernel_spmd(nc, in_maps, list(range(N_CORES)))
    outs = [res.results[i]["out"] for i in range(N_CORES)]
    return np.concatenate(outs, axis=1)
